# revision 19
# baseline (speedup 1.0000x reference)
"""Trainium2 Bass kernel for fused multi-head attention (CompositeMHA).

Reference computation (B=4, S=1024, E=2048, H=16, D=128), fp32:
    proj = x @ in_proj_weight.T + in_proj_bias        # [B,S,3E]
    q,k,v = split(proj); heads of D=128
    ctx = softmax(q k^T / sqrt(D)) v                   # per (b, head)
    out = ctx @ out_proj_weight.T + out_proj_bias      # [B,S,E]

Sharding (8 cores, no on-device collectives), per the tensor-parallel hint:
data-parallel over the 4 batches x tensor-parallel over head halves.
Core c handles batch c//2 and heads [hh*8, hh*8+8) where hh = c%2 —
sharding the corresponding 3E rows of in_proj_weight and columns (e-rows
of W^T) of out_proj_weight.  Each core emits a partial output
[S, E] = ctx_half @ out_proj_weight_half^T (+ bias on the hh==0 core);
the gather step sums each batch's two partials (the TP reduction).
Per-core work is exactly total/8 = 21.5 GFLOP, no duplication.

On-core dataflow (bf16 matmuls into fp32 PSUM):
    xT   = X_b^T                      [E, S]
    K^T[h] = Wk^T-chunk^T @ xT        [D, S]   per head, + bias via DVE
    Q^T[h] likewise                   [D, S]   (1/sqrt(D) folded into Wq,bq)
    V      = xT-chunk^T @ Wv^T        [S, E/2] natural layout, + bias (DVE)
    scoresT[k,q] = K^T-chunk^T @ Q^T  -> exp on ACT -> P^T (bf16)
    sums[q] = ones^T @ P^T (PE); recip (DVE); replicate (GPSIMD)
    ctx^T[h] = V-chunk^T @ P^T accumulated; * recip -> bf16
    out_partial = ctx^T-chunk^T @ Wout^T-half (+ bias) -> fp32
"""

import numpy as np
import ml_dtypes

B, S, E, H = 4, 1024, 2048, 16
D = 128          # head dim == partition size
P = 128
HH = 8           # heads per core (head half)
EH = HH * D      # 1024: e-columns of this half
EC = E // P      # 16 e-chunks (contraction for in-proj)
OC = EH // P     # 8 e-chunks (contraction for out-proj)
NCORES = 8
BF16 = ml_dtypes.bfloat16

_PROGRAM = None


def _build_program(bench_iters=None, phase="full"):
    import concourse.bass as bass  # noqa: F401
    import concourse.tile as tile
    from concourse import bacc, mybir
    from contextlib import ExitStack

    dt = mybir.dt
    AFT = mybir.ActivationFunctionType

    nc = bacc.Bacc("TRN2", target_bir_lowering=False, debug=False,
                   num_devices=NCORES)

    xT_d = nc.dram_tensor("xT", [E, S], dt.bfloat16, kind="ExternalInput").ap()
    wqT_d = nc.dram_tensor("wqT", [E, EH], dt.bfloat16, kind="ExternalInput").ap()
    wkT_d = nc.dram_tensor("wkT", [E, EH], dt.bfloat16, kind="ExternalInput").ap()
    wvT_d = nc.dram_tensor("wvT", [E, EH], dt.bfloat16, kind="ExternalInput").ap()
    woT_d = nc.dram_tensor("woT", [EH, E], dt.bfloat16, kind="ExternalInput").ap()
    bqT_d = nc.dram_tensor("bqT", [P, HH], dt.float32, kind="ExternalInput").ap()
    bkT_d = nc.dram_tensor("bkT", [P, HH], dt.float32, kind="ExternalInput").ap()
    bv_d = nc.dram_tensor("bv", [1, EH], dt.float32, kind="ExternalInput").ap()
    bo_d = nc.dram_tensor("bo", [1, E], dt.float32, kind="ExternalInput").ap()
    out_d = nc.dram_tensor("out", [S, E], dt.float32, kind="ExternalOutput").ap()

    with tile.TileContext(nc) as tc, ExitStack() as ctx:
        sb = ctx.enter_context(tc.tile_pool(name="persist", bufs=1))
        wp = ctx.enter_context(tc.tile_pool(name="wstream", bufs=3))
        ktp = ctx.enter_context(tc.tile_pool(name="ktp", bufs=8))
        qtp = ctx.enter_context(tc.tile_pool(name="qtp", bufs=8))
        ptp = ctx.enter_context(tc.tile_pool(name="ptp", bufs=4))
        outp = ctx.enter_context(tc.tile_pool(name="outp", bufs=3))
        rowp = ctx.enter_context(tc.tile_pool(name="rowp", bufs=2))
        # PSUM: pp(2) + sp(2x2 banks) + cp(1) + su(1) = 8 banks
        ppp = ctx.enter_context(tc.tile_pool(name="ppsum", bufs=2, space="PSUM"))
        spp = ctx.enter_context(tc.tile_pool(name="spsum", bufs=2, space="PSUM"))
        cpp = ctx.enter_context(tc.tile_pool(name="cpsum", bufs=1, space="PSUM"))
        upp = ctx.enter_context(tc.tile_pool(name="upsum", bufs=1, space="PSUM"))

        def emit():
            # ---- persistent loads ----
            xt = []
            for ec in range(EC):
                t = sb.tile([P, S], dt.bfloat16, name=f"xt{ec}", tag=f"xt{ec}")
                nc.sync.dma_start(t[:], xT_d[ec * P:(ec + 1) * P, :])
                xt.append(t)
            bqt = sb.tile([P, HH], dt.float32, name="bqt", tag="bqt")
            nc.sync.dma_start(bqt[:], bqT_d[:])
            bkt = sb.tile([P, HH], dt.float32, name="bkt", tag="bkt")
            nc.sync.dma_start(bkt[:], bkT_d[:])
            bv_rep = sb.tile([P, EH], dt.float32, name="bv_rep", tag="bv_rep")
            nc.sync.dma_start(bv_rep[:], bv_d.to_broadcast((P, EH)))
            bo_rep = sb.tile([P, E], dt.float32, name="bo_rep", tag="bo_rep")
            nc.sync.dma_start(bo_rep[:], bo_d.to_broadcast((P, E)))
            ones_col = sb.tile([P, 1], dt.bfloat16, name="ones_col",
                               tag="ones_col")
            nc.vector.memset(ones_col[:], 1.0)

            v_sb = []
            for sc in range(S // P):
                t = sb.tile([P, EH], dt.bfloat16, name=f"v{sc}", tag=f"v{sc}")
                v_sb.append(t)
            ctxT = []
            for h in range(HH):
                t = sb.tile([P, S], dt.bfloat16, name=f"ctxT{h}",
                            tag=f"ctxT{h}")
                ctxT.append(t)

            def load_w_tiles(dram, col0, label, nec=EC):
                tiles = []
                for ec in range(nec):
                    t = wp.tile([P, 512], dt.bfloat16,
                                name=f"{label}{ec}", tag=f"w{ec}")
                    nc.sync.dma_start(
                        t[:], dram[ec * P:(ec + 1) * P, col0:col0 + 512])
                    tiles.append(t)
                return tiles

            # ---- per-half pipeline: project 4 heads' K/Q/V, then their
            # ---- attention (keeps pool consumer distances short) ----
            KC = S // P  # 8 key chunks
            kt = {}
            qt = {}
            for grp in range(2):
                # K^T for heads 4*grp .. 4*grp+3 (local head index)
                wk = load_w_tiles(wkT_d, grp * 512, f"wk{grp}")
                for hh4 in range(4):
                    h = grp * 4 + hh4
                    kth = ktp.tile([P, S], dt.bfloat16, name=f"kt{h}",
                                   tag="kt")
                    for sb2 in range(S // 512):
                        ps = ppp.tile([P, 512], dt.float32, name="kps",
                                      tag="pp")
                        for ec in range(EC):
                            nc.tensor.matmul(
                                ps[:],
                                wk[ec][:, hh4 * P:(hh4 + 1) * P],
                                xt[ec][:, sb2 * 512:(sb2 + 1) * 512],
                                start=(ec == 0), stop=(ec == EC - 1))
                        nc.vector.tensor_scalar_add(
                            kth[:, sb2 * 512:(sb2 + 1) * 512], ps[:],
                            bkt[:, h:h + 1])
                    kt[h] = kth

                # Q^T for the same heads
                wq = load_w_tiles(wqT_d, grp * 512, f"wq{grp}")
                for hh4 in range(4):
                    h = grp * 4 + hh4
                    qth = qtp.tile([P, S], dt.bfloat16, name=f"qt{h}",
                                   tag="qt")
                    for sb2 in range(S // 512):
                        ps = ppp.tile([P, 512], dt.float32, name="qps",
                                      tag="pp")
                        for ec in range(EC):
                            nc.tensor.matmul(
                                ps[:],
                                wq[ec][:, hh4 * P:(hh4 + 1) * P],
                                xt[ec][:, sb2 * 512:(sb2 + 1) * 512],
                                start=(ec == 0), stop=(ec == EC - 1))
                        nc.vector.tensor_scalar_add(
                            qth[:, sb2 * 512:(sb2 + 1) * 512], ps[:],
                            bqt[:, h:h + 1])
                    qt[h] = qth

                # V columns for these 4 heads (natural [s, e] layout)
                fw = grp
                wv = load_w_tiles(wvT_d, fw * 512, f"wv{grp}")
                for sc in range(S // P):
                    ps = ppp.tile([P, 512], dt.float32, name="vps", tag="pp")
                    for ec in range(EC):
                        nc.tensor.matmul(
                            ps[:],
                            xt[ec][:, sc * P:(sc + 1) * P],
                            wv[ec][:],
                            start=(ec == 0), stop=(ec == EC - 1))
                    nc.vector.tensor_add(
                        v_sb[sc][:, fw * 512:(fw + 1) * 512], ps[:],
                        bv_rep[:, fw * 512:(fw + 1) * 512])

                # attention for heads 4*grp .. 4*grp+3, q in two 512-blocks
                for hh4 in range(4 if phase != "proj" else 0):
                    h = grp * 4 + hh4
                    for qb in range(S // 512):
                        qsl = slice(qb * 512, (qb + 1) * 512)
                        sums_ps = upp.tile([1, 512], dt.float32,
                                           name=f"sums{h}_{qb}", tag="su")
                        ctx_ps = cpp.tile([P, 512], dt.float32,
                                          name=f"ctxps{h}_{qb}", tag="cp")
                        # two k-chunks share one 2-bank psum tile so each
                        # exp covers 1024 cols (ACT has ~300ns/op fixed cost)
                        for kc2 in range(KC // 2):
                            sps = spp.tile([P, 1024], dt.float32, name="sps",
                                           tag="sp")
                            pt = ptp.tile([P, 1024], dt.bfloat16, name="pt",
                                          tag="pt")
                            for j in range(2):
                                kc = 2 * kc2 + j
                                nc.tensor.matmul(
                                    sps[:, j * 512:(j + 1) * 512],
                                    kt[h][:, kc * P:(kc + 1) * P],
                                    qt[h][:, qsl], start=True, stop=True)
                            nc.scalar.activation(pt[:], sps[:], AFT.Exp)
                            for j in range(2):
                                kc = 2 * kc2 + j
                                jsl = slice(j * 512, (j + 1) * 512)
                                nc.tensor.matmul(sums_ps[:], ones_col[:],
                                                 pt[:, jsl],
                                                 start=(kc == 0),
                                                 stop=(kc == KC - 1))
                                nc.tensor.matmul(
                                    ctx_ps[:],
                                    v_sb[kc][:, h * P:(h + 1) * P],
                                    pt[:, jsl], start=(kc == 0),
                                    stop=(kc == KC - 1))
                        r_row = rowp.tile([1, 512], dt.float32, name="r_row",
                                          tag="rrow")
                        nc.vector.reciprocal(r_row[:], sums_ps[:])
                        rep = rowp.tile([P, 512], dt.float32, name="rep",
                                        tag="rep")
                        nc.gpsimd.partition_broadcast(rep[:], r_row[:])
                        nc.vector.tensor_mul(ctxT[h][:, qsl], ctx_ps[:],
                                             rep[:])

            # ---- output projection (partial: contracts this e-half) ----
            for fw in range(4 if phase == "full" else 0):
                wo = load_w_tiles(woT_d, fw * 512, f"wo{fw}", nec=OC)
                for qc in range(S // P):
                    ps = ppp.tile([P, 512], dt.float32, name="ops", tag="pp")
                    for h in range(HH):
                        nc.tensor.matmul(
                            ps[:],
                            ctxT[h][:, qc * P:(qc + 1) * P],
                            wo[h][:],
                            start=(h == 0), stop=(h == HH - 1))
                    osb = outp.tile([P, 512], dt.float32, name="osb",
                                    tag="ot")
                    nc.vector.tensor_add(
                        osb[:], ps[:], bo_rep[:, fw * 512:(fw + 1) * 512])
                    nc.sync.dma_start(
                        out_d[qc * P:(qc + 1) * P,
                              fw * 512:(fw + 1) * 512], osb[:])

        if bench_iters is None:
            emit()
        else:
            with tc.For_i(0, bench_iters, 1):
                emit()

    nc.compile()
    return nc


def _get_program():
    global _PROGRAM
    if _PROGRAM is None:
        _PROGRAM = _build_program()
    return _PROGRAM


def make_in_maps(query, in_proj_weight, in_proj_bias, out_proj_weight,
                 out_proj_bias):
    """Host-side sharding: slice/transpose/cast per core. Pure layout prep."""
    x = np.asarray(query, dtype=np.float32)
    W = np.asarray(in_proj_weight, dtype=np.float32)
    b = np.asarray(in_proj_bias, dtype=np.float32)
    Wo = np.asarray(out_proj_weight, dtype=np.float32)
    bo = np.asarray(out_proj_bias, dtype=np.float32)

    sc = np.float32(1.0 / np.sqrt(D))
    wqT = np.ascontiguousarray((W[:E] * sc).T).astype(BF16)       # [E, E]
    wkT = np.ascontiguousarray(W[E:2 * E].T).astype(BF16)
    wvT = np.ascontiguousarray(W[2 * E:].T).astype(BF16)
    woT = np.ascontiguousarray(Wo.T).astype(BF16)                 # [E, E]
    bq_s = (b[:E] * sc).reshape(H, P)
    bk_s = b[E:2 * E].reshape(H, P)
    bv_s = b[2 * E:].reshape(1, E)
    bo_r = np.ascontiguousarray(bo.reshape(1, E))
    bo_zero = np.zeros_like(bo_r)

    in_maps = []
    for c in range(NCORES):
        bi, hh = c // 2, c % 2
        esl = slice(hh * EH, (hh + 1) * EH)
        xT = np.ascontiguousarray(x[bi].T).astype(BF16)
        in_maps.append({
            "xT": xT,
            "wqT": np.ascontiguousarray(wqT[:, esl]),
            "wkT": np.ascontiguousarray(wkT[:, esl]),
            "wvT": np.ascontiguousarray(wvT[:, esl]),
            "woT": np.ascontiguousarray(woT[esl, :]),
            "bqT": np.ascontiguousarray(bq_s[hh * HH:(hh + 1) * HH].T),
            "bkT": np.ascontiguousarray(bk_s[hh * HH:(hh + 1) * HH].T),
            "bv": np.ascontiguousarray(bv_s[:, esl]),
            "bo": bo_r if hh == 0 else bo_zero,
        })
    return in_maps


def assemble_out(results):
    """Gather: sum each batch's two tensor-parallel partial outputs."""
    out = np.empty((B, S, E), dtype=np.float32)
    for bi in range(B):
        out[bi] = results[2 * bi]["out"] + results[2 * bi + 1]["out"]
    return out


def kernel(query, in_proj_weight, in_proj_bias, out_proj_weight,
           out_proj_bias):
    from concourse import bass_utils
    nc = _get_program()
    in_maps = make_in_maps(query, in_proj_weight, in_proj_bias,
                           out_proj_weight, out_proj_bias)
    res = bass_utils.run_bass_kernel_spmd(nc, in_maps,
                                          core_ids=list(range(NCORES)))
    return assemble_out(res.results)


# revision 21
# speedup vs baseline: 1.0582x; 1.0582x over previous
"""Trainium2 Bass kernel for fused multi-head attention (CompositeMHA).

Reference computation (B=4, S=1024, E=2048, H=16, D=128), fp32:
    proj = x @ in_proj_weight.T + in_proj_bias        # [B,S,3E]
    q,k,v = split(proj); heads of D=128
    ctx = softmax(q k^T / sqrt(D)) v                   # per (b, head)
    out = ctx @ out_proj_weight.T + out_proj_bias      # [B,S,E]

Sharding (8 cores, no on-device collectives), per the tensor-parallel hint:
data-parallel over the 4 batches x tensor-parallel over head halves.
Core c handles batch c//2 and heads [hh*8, hh*8+8) where hh = c%2 —
sharding the corresponding 3E rows of in_proj_weight and columns (e-rows
of W^T) of out_proj_weight.  Each core emits a partial output
[S, E] = ctx_half @ out_proj_weight_half^T (+ bias on the hh==0 core);
the gather step sums each batch's two partials (the TP reduction).
Per-core work is exactly total/8 = 21.5 GFLOP, no duplication.

On-core dataflow (bf16 matmuls into fp32 PSUM):
    xT   = X_b^T                      [E, S]
    K^T[h] = Wk^T-chunk^T @ xT        [D, S]   per head, + bias via DVE
    Q^T[h] likewise                   [D, S]   (1/sqrt(D) folded into Wq,bq)
    V      = xT-chunk^T @ Wv^T        [S, E/2] natural layout, + bias (DVE)
    scoresT[k,q] = K^T-chunk^T @ Q^T  -> exp on ACT -> P^T (bf16)
    sums[q] = ones^T @ P^T (PE); recip (DVE); replicate (GPSIMD)
    ctx^T[h] = V-chunk^T @ P^T accumulated; * recip -> bf16
    out_partial = ctx^T-chunk^T @ Wout^T-half (+ bias) -> fp32
"""

import numpy as np
import ml_dtypes

B, S, E, H = 4, 1024, 2048, 16
D = 128          # head dim == partition size
P = 128
HH = 8           # heads per core (head half)
EH = HH * D      # 1024: e-columns of this half
EC = E // P      # 16 e-chunks (contraction for in-proj)
OC = EH // P     # 8 e-chunks (contraction for out-proj)
NCORES = 8
BF16 = ml_dtypes.bfloat16

_PROGRAM = None


def _build_program(bench_iters=None, phase="full"):
    import concourse.bass as bass  # noqa: F401
    import concourse.tile as tile
    from concourse import bacc, mybir
    from contextlib import ExitStack

    dt = mybir.dt
    AFT = mybir.ActivationFunctionType

    nc = bacc.Bacc("TRN2", target_bir_lowering=False, debug=False,
                   num_devices=NCORES)

    xT_d = nc.dram_tensor("xT", [E, S], dt.bfloat16, kind="ExternalInput").ap()
    wqT_d = nc.dram_tensor("wqT", [E, EH], dt.bfloat16, kind="ExternalInput").ap()
    wkT_d = nc.dram_tensor("wkT", [E, EH], dt.bfloat16, kind="ExternalInput").ap()
    wvT_d = nc.dram_tensor("wvT", [E, EH], dt.bfloat16, kind="ExternalInput").ap()
    woT_d = nc.dram_tensor("woT", [EH, E], dt.bfloat16, kind="ExternalInput").ap()
    bqT_d = nc.dram_tensor("bqT", [P, HH], dt.float32, kind="ExternalInput").ap()
    bkT_d = nc.dram_tensor("bkT", [P, HH], dt.float32, kind="ExternalInput").ap()
    bv_d = nc.dram_tensor("bv", [1, EH], dt.float32, kind="ExternalInput").ap()
    bo_d = nc.dram_tensor("bo", [1, E], dt.float32, kind="ExternalInput").ap()
    out_d = nc.dram_tensor("out", [S, E], dt.float32, kind="ExternalOutput").ap()

    with tile.TileContext(nc) as tc, ExitStack() as ctx:
        sb = ctx.enter_context(tc.tile_pool(name="persist", bufs=1))
        wp = ctx.enter_context(tc.tile_pool(name="wstream", bufs=3))
        ktp = ctx.enter_context(tc.tile_pool(name="ktp", bufs=8))
        qtp = ctx.enter_context(tc.tile_pool(name="qtp", bufs=8))
        ptp = ctx.enter_context(tc.tile_pool(name="ptp", bufs=4))
        outp = ctx.enter_context(tc.tile_pool(name="outp", bufs=3))
        rowp = ctx.enter_context(tc.tile_pool(name="rowp", bufs=2))
        # PSUM: pp(2) + sp(3) + cp(2) + su(1) = 8 banks
        ppp = ctx.enter_context(tc.tile_pool(name="ppsum", bufs=2, space="PSUM"))
        spp = ctx.enter_context(tc.tile_pool(name="spsum", bufs=3, space="PSUM"))
        cpp = ctx.enter_context(tc.tile_pool(name="cpsum", bufs=2, space="PSUM"))
        upp = ctx.enter_context(tc.tile_pool(name="upsum", bufs=1, space="PSUM"))

        def emit():
            # ---- persistent loads ----
            xt = []
            for ec in range(EC):
                t = sb.tile([P, S], dt.bfloat16, name=f"xt{ec}", tag=f"xt{ec}")
                nc.sync.dma_start(t[:], xT_d[ec * P:(ec + 1) * P, :])
                xt.append(t)
            bqt = sb.tile([P, HH], dt.float32, name="bqt", tag="bqt")
            nc.sync.dma_start(bqt[:], bqT_d[:])
            bkt = sb.tile([P, HH], dt.float32, name="bkt", tag="bkt")
            nc.sync.dma_start(bkt[:], bkT_d[:])
            bv_rep = sb.tile([P, EH], dt.float32, name="bv_rep", tag="bv_rep")
            nc.sync.dma_start(bv_rep[:], bv_d.to_broadcast((P, EH)))
            bo_rep = sb.tile([P, E], dt.float32, name="bo_rep", tag="bo_rep")
            nc.sync.dma_start(bo_rep[:], bo_d.to_broadcast((P, E)))
            ones_col = sb.tile([P, 1], dt.bfloat16, name="ones_col",
                               tag="ones_col")
            nc.vector.memset(ones_col[:], 1.0)

            v_sb = []
            for sc in range(S // P):
                t = sb.tile([P, EH], dt.bfloat16, name=f"v{sc}", tag=f"v{sc}")
                v_sb.append(t)
            ctxT = []
            for h in range(HH):
                t = sb.tile([P, S], dt.bfloat16, name=f"ctxT{h}",
                            tag=f"ctxT{h}")
                ctxT.append(t)

            def load_w_tiles(dram, col0, label, nec=EC):
                tiles = []
                for ec in range(nec):
                    t = wp.tile([P, 512], dt.bfloat16,
                                name=f"{label}{ec}", tag=f"w{ec}")
                    nc.sync.dma_start(
                        t[:], dram[ec * P:(ec + 1) * P, col0:col0 + 512])
                    tiles.append(t)
                return tiles

            # ---- per-half pipeline: project 4 heads' K/Q/V, then their
            # ---- attention (keeps pool consumer distances short) ----
            KC = S // P  # 8 key chunks
            kt = {}
            qt = {}
            for grp in range(2):
                # K^T for heads 4*grp .. 4*grp+3 (local head index)
                wk = load_w_tiles(wkT_d, grp * 512, f"wk{grp}")
                for hh4 in range(4):
                    h = grp * 4 + hh4
                    kth = ktp.tile([P, S], dt.bfloat16, name=f"kt{h}",
                                   tag="kt")
                    for sb2 in range(S // 512):
                        ps = ppp.tile([P, 512], dt.float32, name="kps",
                                      tag="pp")
                        for ec in range(EC):
                            nc.tensor.matmul(
                                ps[:],
                                wk[ec][:, hh4 * P:(hh4 + 1) * P],
                                xt[ec][:, sb2 * 512:(sb2 + 1) * 512],
                                start=(ec == 0), stop=(ec == EC - 1))
                        nc.vector.tensor_scalar_add(
                            kth[:, sb2 * 512:(sb2 + 1) * 512], ps[:],
                            bkt[:, h:h + 1])
                    kt[h] = kth

                # Q^T for the same heads
                wq = load_w_tiles(wqT_d, grp * 512, f"wq{grp}")
                for hh4 in range(4):
                    h = grp * 4 + hh4
                    qth = qtp.tile([P, S], dt.bfloat16, name=f"qt{h}",
                                   tag="qt")
                    for sb2 in range(S // 512):
                        ps = ppp.tile([P, 512], dt.float32, name="qps",
                                      tag="pp")
                        for ec in range(EC):
                            nc.tensor.matmul(
                                ps[:],
                                wq[ec][:, hh4 * P:(hh4 + 1) * P],
                                xt[ec][:, sb2 * 512:(sb2 + 1) * 512],
                                start=(ec == 0), stop=(ec == EC - 1))
                        nc.vector.tensor_scalar_add(
                            qth[:, sb2 * 512:(sb2 + 1) * 512], ps[:],
                            bqt[:, h:h + 1])
                    qt[h] = qth

                # V columns for these 4 heads (natural [s, e] layout)
                fw = grp
                wv = load_w_tiles(wvT_d, fw * 512, f"wv{grp}")
                for sc in range(S // P):
                    ps = ppp.tile([P, 512], dt.float32, name="vps", tag="pp")
                    for ec in range(EC):
                        nc.tensor.matmul(
                            ps[:],
                            xt[ec][:, sc * P:(sc + 1) * P],
                            wv[ec][:],
                            start=(ec == 0), stop=(ec == EC - 1))
                    nc.vector.tensor_add(
                        v_sb[sc][:, fw * 512:(fw + 1) * 512], ps[:],
                        bv_rep[:, fw * 512:(fw + 1) * 512])

                # attention for heads 4*grp .. 4*grp+3, q in two 512-blocks
                for hh4 in range(4 if phase != "proj" else 0):
                    h = grp * 4 + hh4
                    for qb in range(S // 512):
                        qsl = slice(qb * 512, (qb + 1) * 512)
                        sums_ps = upp.tile([1, 512], dt.float32,
                                           name=f"sums{h}_{qb}", tag="su")
                        ctx_ps = cpp.tile([P, 512], dt.float32,
                                          name=f"ctxps{h}_{qb}", tag="cp")
                        for kc in range(KC):
                            sps = spp.tile([P, 512], dt.float32, name="sps",
                                           tag="sp")
                            nc.tensor.matmul(sps[:],
                                             kt[h][:, kc * P:(kc + 1) * P],
                                             qt[h][:, qsl],
                                             start=True, stop=True)
                            pt = ptp.tile([P, 512], dt.bfloat16, name="pt",
                                          tag="pt")
                            nc.scalar.activation(pt[:], sps[:], AFT.Exp)
                            nc.tensor.matmul(sums_ps[:], ones_col[:], pt[:],
                                             start=(kc == 0),
                                             stop=(kc == KC - 1))
                            nc.tensor.matmul(
                                ctx_ps[:],
                                v_sb[kc][:, h * P:(h + 1) * P],
                                pt[:], start=(kc == 0), stop=(kc == KC - 1))
                        r_row = rowp.tile([1, 512], dt.float32, name="r_row",
                                          tag="rrow")
                        nc.vector.reciprocal(r_row[:], sums_ps[:])
                        rep = rowp.tile([P, 512], dt.float32, name="rep",
                                        tag="rep")
                        nc.gpsimd.partition_broadcast(rep[:], r_row[:])
                        nc.vector.tensor_mul(ctxT[h][:, qsl], ctx_ps[:],
                                             rep[:])

            # ---- output projection (partial: contracts this e-half) ----
            for fw in range(4 if phase == "full" else 0):
                wo = load_w_tiles(woT_d, fw * 512, f"wo{fw}", nec=OC)
                for qc in range(S // P):
                    ps = ppp.tile([P, 512], dt.float32, name="ops", tag="pp")
                    for h in range(HH):
                        nc.tensor.matmul(
                            ps[:],
                            ctxT[h][:, qc * P:(qc + 1) * P],
                            wo[h][:],
                            start=(h == 0), stop=(h == HH - 1))
                    osb = outp.tile([P, 512], dt.float32, name="osb",
                                    tag="ot")
                    nc.vector.tensor_add(
                        osb[:], ps[:], bo_rep[:, fw * 512:(fw + 1) * 512])
                    nc.sync.dma_start(
                        out_d[qc * P:(qc + 1) * P,
                              fw * 512:(fw + 1) * 512], osb[:])

        if bench_iters is None:
            emit()
        else:
            with tc.For_i(0, bench_iters, 1):
                emit()

    nc.compile()
    return nc


def _get_program():
    global _PROGRAM
    if _PROGRAM is None:
        _PROGRAM = _build_program()
    return _PROGRAM


def make_in_maps(query, in_proj_weight, in_proj_bias, out_proj_weight,
                 out_proj_bias):
    """Host-side sharding: slice/transpose/cast per core. Pure layout prep."""
    x = np.asarray(query, dtype=np.float32)
    W = np.asarray(in_proj_weight, dtype=np.float32)
    b = np.asarray(in_proj_bias, dtype=np.float32)
    Wo = np.asarray(out_proj_weight, dtype=np.float32)
    bo = np.asarray(out_proj_bias, dtype=np.float32)

    sc = np.float32(1.0 / np.sqrt(D))
    wqT = np.ascontiguousarray((W[:E] * sc).T).astype(BF16)       # [E, E]
    wkT = np.ascontiguousarray(W[E:2 * E].T).astype(BF16)
    wvT = np.ascontiguousarray(W[2 * E:].T).astype(BF16)
    woT = np.ascontiguousarray(Wo.T).astype(BF16)                 # [E, E]
    bq_s = (b[:E] * sc).reshape(H, P)
    bk_s = b[E:2 * E].reshape(H, P)
    bv_s = b[2 * E:].reshape(1, E)
    bo_r = np.ascontiguousarray(bo.reshape(1, E))
    bo_zero = np.zeros_like(bo_r)

    in_maps = []
    for c in range(NCORES):
        bi, hh = c // 2, c % 2
        esl = slice(hh * EH, (hh + 1) * EH)
        xT = np.ascontiguousarray(x[bi].T).astype(BF16)
        in_maps.append({
            "xT": xT,
            "wqT": np.ascontiguousarray(wqT[:, esl]),
            "wkT": np.ascontiguousarray(wkT[:, esl]),
            "wvT": np.ascontiguousarray(wvT[:, esl]),
            "woT": np.ascontiguousarray(woT[esl, :]),
            "bqT": np.ascontiguousarray(bq_s[hh * HH:(hh + 1) * HH].T),
            "bkT": np.ascontiguousarray(bk_s[hh * HH:(hh + 1) * HH].T),
            "bv": np.ascontiguousarray(bv_s[:, esl]),
            "bo": bo_r if hh == 0 else bo_zero,
        })
    return in_maps


def assemble_out(results):
    """Gather: sum each batch's two tensor-parallel partial outputs."""
    out = np.empty((B, S, E), dtype=np.float32)
    for bi in range(B):
        out[bi] = results[2 * bi]["out"] + results[2 * bi + 1]["out"]
    return out


def kernel(query, in_proj_weight, in_proj_bias, out_proj_weight,
           out_proj_bias):
    from concourse import bass_utils
    nc = _get_program()
    in_maps = make_in_maps(query, in_proj_weight, in_proj_bias,
                           out_proj_weight, out_proj_bias)
    res = bass_utils.run_bass_kernel_spmd(nc, in_maps,
                                          core_ids=list(range(NCORES)))
    return assemble_out(res.results)
